# revision 10
# baseline (speedup 1.0000x reference)
"""Trainium2 Bass kernel for nn_Cholesky_from_z (pair-compressed, merged-ln).

Closed form: L[i,j] = z[i,j] * sqrt(prod_{k<j}(1-z[i,k]^2)) (j<i),
L[i,i] = sqrt(prod_{k<i}(...)) -- exclusive cumprod of a=(1-z^2) along each
matrix row, with the serial scan replaced by a matmul on the tensor engine
and positions PAIR-COMPRESSED.

v6 structure (one table swap, merged single Ln):
  phase A [sqrt table]:  ue = ze*ze (DVE); sqe = Sqrt(1-ue) (ACT)
  swap to ln/exp table
  phase B: uo = zo*zo; buen = ue-1; buon = uo-1; pp = buen*buon   DVE
           w  = Ln(pp)          ACT   (pp = (1-ze^2)(1-zo^2) >= 0)
           C  = S^T @ w         PE    (S fp8 {0,1}, per pair-block)
           ge = Exp(0.5*C)      ACT
           zs = zo*sqe; le = ze*ge; lo = zs*ge                    DVE
Diagonal sentinel z=0.998 (keeps Ln finite); host divides diag by it.
Batch 2048 sharded 256 samples/core over 8 cores; fp16 I/O.
PE is pre-warmed with junk matmuls during phase A so the HAM throttle is
at full rate when the real matmuls arrive.
"""

import sys

if "/opt/trn_rl_repo" not in sys.path:
    sys.path.insert(0, "/opt/trn_rl_repo")

import numpy as np

B = 2048
N = 128
NZ = N * (N - 1) // 2          # 8128
NBLK = 65
PACK = NBLK * 128              # 8320 (even-aligned rows)
NPAIR = PACK // 2              # 4160
PBLK = 33
PPAD = PBLK * 128              # 4224
NCORES = 8
SAMP = B // NCORES             # 256
SENT = np.float16(0.998)
CHUNKS = [(0, 2), (2, 8), (8, 14), (14, 20), (20, 26), (26, 32), (32, PBLK)]
GPS_ZS = 3                     # chunks of zs = zo*sqe computed on GPSIMD
WARMUP_PER_CHUNK = 3           # junk matmuls to warm the PE HAM throttle

# --- host-side packing maps -------------------------------------------------
def _build_maps():
    row_of_block = [(i, 125 - i) for i in range(63)] + [(126,), (127,)]
    slot_row = np.full(PACK, -1, np.int64)
    slot_col = np.full(PACK, -1, np.int64)
    for b, rows in enumerate(row_of_block):
        pos = b * 128
        for r in rows:
            L = r + 1
            slot_row[pos:pos + L] = r
            slot_col[pos:pos + L - 1] = np.arange(r)
            slot_col[pos + L - 1] = r              # diag/sentinel slot
            pos += L + (1 if r % 2 == 0 else 0)    # pad rows to even length
    return slot_row, slot_col

_slot_row, _slot_col = _build_maps()
_valid = _slot_row >= 0
_strict = _valid & (_slot_col < _slot_row)
_diag = _valid & (_slot_col == _slot_row)
_tri_idx = (_slot_row[_strict] * (_slot_row[_strict] - 1) // 2
            + _slot_col[_strict])

def _build_S():
    from ml_dtypes import float8_e4m3
    pr = np.full(PPAD, -1, np.int64)
    pr[:NPAIR] = _slot_row[0::2]
    S = np.zeros((PBLK, 128, 128), np.float32)
    k = np.arange(128)
    for q in range(PBLK):
        s = pr[q * 128:(q + 1) * 128]
        same = (s[:, None] == s[None, :]) & (s[:, None] >= 0)
        S[q] = np.where(same & (k[:, None] < k[None, :]), 1.0, 0.0)
    return np.ascontiguousarray(
        S.transpose(1, 0, 2)).astype(float8_e4m3)   # [k, blk, t]

_S_host = _build_S()

_prog_cache = {}


def _build_program():
    import concourse.bacc as bacc
    import concourse.mybir as mybir
    import bass_rust
    from concourse.tile import TileContext

    f16 = mybir.dt.float16
    f32 = mybir.dt.float32
    f8 = mybir.dt.float8e4
    Act = mybir.ActivationFunctionType
    Alu = mybir.AluOpType

    nc = bacc.Bacc("TRN2", target_bir_lowering=False, debug=False,
                   num_devices=NCORES)

    # The act-table placement pass maps each activation func to the FIRST
    # set containing it (ln->natural_log, exp->exp_and_others), forcing a
    # 1.28us table reload on every ln<->exp transition. Filter the table
    # list it sees so ln/exp resolve only to natural_log_exp_and_others
    # and sqrt to sqrt_and_others; list order (and thus the emitted
    # act_func_set_id indices) is unchanged, so the real tables load.
    import types
    from concourse.hw_specs import get_activation_tables

    def _patched_table_loads(self):
        has_activation = any(
            isinstance(i, mybir.InstActivation)
            for b in self.main_func.blocks
            for i in b.instructions
        )
        if not has_activation:
            return
        keep = {"sqrt_and_others", "natural_log_exp_and_others"}
        drop = {Act.Ln, Act.Exp, Act.Sqrt}
        tables = [
            (name, fns if name in keep else fns - drop)
            for name, fns in get_activation_tables(self.m.arch).items()
        ]
        bass_rust.insert_act_table_loads(self, tables)

    nc.insert_act_table_loads = types.MethodType(_patched_table_loads, nc)
    ze_d = nc.dram_tensor("ze", [128, PBLK, SAMP], f16,
                          kind="ExternalInput").ap()
    zo_d = nc.dram_tensor("zo", [128, PBLK, SAMP], f16,
                          kind="ExternalInput").ap()
    sc_d = nc.dram_tensor("sc", [128, PBLK, 128], f8,
                          kind="ExternalInput").ap()
    lp_d = nc.dram_tensor("lp", [128, PBLK, 2, SAMP], f16,
                          kind="ExternalOutput").ap()

    def load_table(set_id):
        _tl = bass_rust.InstLoadActFuncSet(
            name=nc.get_next_instruction_name(), ins=[], outs=[],
            act_func_set_id=set_id)
        nc.scalar.add_instruction(_tl)

    with TileContext(nc) as tc:
        with (
            tc.tile_pool(name="sb", bufs=1) as sb,
            tc.psum_pool(name="ps", bufs=2) as pp_pool,
            tc.psum_pool(name="junk", bufs=1) as junk_pool,
        ):
            ze = sb.tile([128, PBLK, SAMP], f16)
            zo = sb.tile([128, PBLK, SAMP], f16)
            uez = sb.tile([128, PBLK, SAMP], f16)    # ue, later zs
            uo = sb.tile([128, PBLK, SAMP], f16)
            buw = sb.tile([128, PBLK, SAMP], f16)    # buen, later w
            buon = sb.tile([128, PBLK, SAMP], f16)
            ppt = sb.tile([128, PBLK, SAMP], f16)
            sqe = sb.tile([128, PBLK, SAMP], f16)
            ge = sb.tile([128, PBLK, SAMP], f16)
            lt = sb.tile([128, PBLK, 2, SAMP], f16)
            st = sb.tile([128, PBLK, 128], f8)

            # ---- phase A: ze in, ue, sqe (sqrt table) ----
            for a, b in CHUNKS:
                g = (slice(None), slice(a, b), slice(None))
                nc.sync.dma_start(out=ze[g], in_=ze_d[g])
            jt = junk_pool.tile([128, 2, SAMP], f32, tag="junk")
            for i, (a, b) in enumerate(CHUNKS):
                g = (slice(None), slice(a, b), slice(None))
                nc.vector.tensor_mul(uez[g], ze[g], ze[g])
                nc.scalar.activation(sqe[g], uez[g], Act.Sqrt,
                                     bias=1.0, scale=-1.0)
                # PE warmup: junk matmuls on the freshly-arrived ze chunk
                for k in range(WARMUP_PER_CHUNK):
                    nc.tensor.matmul(jt[:, k % 2, :], ze[:, a, 0:128],
                                     ze[:, a, :])

            # ---- phase B ----
            nc.sync.dma_start(out=st[:], in_=sc_d)
            for a, b in CHUNKS:
                g = (slice(None), slice(a, b), slice(None))
                nc.sync.dma_start(out=zo[g], in_=zo_d[g])

            def emit_batch(a, b):
                nb = b - a
                pt = pp_pool.tile([128, 6, SAMP], f32, tag="ps")
                for j in range(nb):
                    nc.tensor.matmul(pt[:, j, :], st[:, a + j, :],
                                     buw[:, a + j, :])
                gsl = (slice(None), slice(a, b), slice(None))
                nc.scalar.activation(ge[gsl], pt[:, 0:nb, :], Act.Exp,
                                     scale=0.5)
                nc.vector.tensor_mul(lt[:, a:b, 0, :], ze[gsl], ge[gsl])
                nc.vector.tensor_mul(lt[:, a:b, 1, :], uez[gsl], ge[gsl])
                nc.sync.dma_start(out=lp_d[:, a:b, :, :],
                                  in_=lt[:, a:b, :, :])

            prev = None
            for i, (a, b) in enumerate(CHUNKS):
                g = (slice(None), slice(a, b), slice(None))
                nc.vector.tensor_mul(uo[g], zo[g], zo[g])
                nc.vector.tensor_scalar(buw[g], uez[g], 1.0, None,
                                        Alu.subtract)        # ue - 1
                nc.vector.tensor_scalar(buon[g], uo[g], 1.0, None,
                                        Alu.subtract)        # uo - 1
                nc.vector.tensor_mul(ppt[g], buw[g], buon[g])
                # zs = zo*sqe overwrites ue (dead after buen/sqe)
                if i < GPS_ZS:
                    nc.gpsimd.tensor_tensor(uez[g], zo[g], sqe[g], Alu.mult)
                else:
                    nc.vector.tensor_mul(uez[g], zo[g], sqe[g])
                nc.scalar.activation(buw[g], ppt[g], Act.Ln)  # w over buen
                if prev is not None:
                    emit_batch(*prev)
                prev = (a, b)
            emit_batch(*prev)
    nc.compile()
    return nc


def _get_program():
    if "nc" not in _prog_cache:
        _prog_cache["nc"] = _build_program()
    return _prog_cache["nc"]


def _to_core(a):
    # [SAMP, PPAD] -> [128, PBLK, SAMP]
    return np.ascontiguousarray(
        a.T.reshape(PBLK, 128, SAMP).transpose(1, 0, 2))


def kernel(inputs: np.ndarray, _return_raw=False, **run_kw) -> np.ndarray:
    from concourse.bass_utils import run_bass_kernel_spmd

    assert inputs.shape == (B, NZ), inputs.shape
    zvec = inputs.astype(np.float16)

    zpk = np.zeros((B, PACK), np.float16)
    zpk[:, _strict] = zvec[:, _tri_idx]
    zpk[:, _diag] = SENT
    ze_all = np.zeros((B, PPAD), np.float16)
    zo_all = np.zeros((B, PPAD), np.float16)
    ze_all[:, :NPAIR] = zpk[:, 0::2]
    zo_all[:, :NPAIR] = zpk[:, 1::2]

    in_maps = []
    for c in range(NCORES):
        sl = slice(c * SAMP, (c + 1) * SAMP)
        in_maps.append({"ze": _to_core(ze_all[sl]),
                        "zo": _to_core(zo_all[sl]),
                        "sc": _S_host})

    nc = _get_program()
    res = run_bass_kernel_spmd(nc, in_maps, list(range(NCORES)), **run_kw)

    lpk = np.zeros((B, PACK), np.float32)
    for c in range(NCORES):
        lc = res.results[c]["lp"]               # [128, PBLK, 2, SAMP]
        le = lc[:, :, 0, :].transpose(1, 0, 2).reshape(PPAD, SAMP).T
        lo = lc[:, :, 1, :].transpose(1, 0, 2).reshape(PPAD, SAMP).T
        sl = slice(c * SAMP, (c + 1) * SAMP)
        lpk[sl, 0::2] = le[:, :NPAIR]
        lpk[sl, 1::2] = lo[:, :NPAIR]

    out = np.zeros((B, N, N), np.float32)
    out[:, _slot_row[_strict], _slot_col[_strict]] = lpk[:, _strict]
    out[:, _slot_row[_diag], _slot_col[_diag]] = (
        lpk[:, _diag] / np.float32(SENT))
    if _return_raw:
        return out, res
    return out


# revision 11
# speedup vs baseline: 1.0445x; 1.0445x over previous
"""Trainium2 Bass kernel for nn_Cholesky_from_z (pair-compressed, merged-ln).

Closed form: L[i,j] = z[i,j] * sqrt(prod_{k<j}(1-z[i,k]^2)) (j<i),
L[i,i] = sqrt(prod_{k<i}(...)) -- exclusive cumprod of a=(1-z^2) along each
matrix row, with the serial scan replaced by a matmul on the tensor engine
and positions PAIR-COMPRESSED.

v6 structure (one table swap, merged single Ln):
  phase A [sqrt table]:  ue = ze*ze (DVE); sqe = Sqrt(1-ue) (ACT)
  swap to ln/exp table
  phase B: uo = zo*zo; buen = ue-1; buon = uo-1; pp = buen*buon   DVE
           w  = Ln(pp)          ACT   (pp = (1-ze^2)(1-zo^2) >= 0)
           C  = S^T @ w         PE    (S fp8 {0,1}, per pair-block)
           ge = Exp(0.5*C)      ACT
           zs = zo*sqe; le = ze*ge; lo = zs*ge                    DVE
Diagonal sentinel z=0.998 (keeps Ln finite); host divides diag by it.
Batch 2048 sharded 256 samples/core over 8 cores; fp16 I/O.
PE is pre-warmed with junk matmuls during phase A so the HAM throttle is
at full rate when the real matmuls arrive.
"""

import sys

if "/opt/trn_rl_repo" not in sys.path:
    sys.path.insert(0, "/opt/trn_rl_repo")

import numpy as np

B = 2048
N = 128
NZ = N * (N - 1) // 2          # 8128
NBLK = 65
PACK = NBLK * 128              # 8320 (even-aligned rows)
NPAIR = PACK // 2              # 4160
PBLK = 33
PPAD = PBLK * 128              # 4224
NCORES = 8
SAMP = B // NCORES             # 256
SENT = np.float16(0.998)
CHUNKS = [(0, 2), (2, 8), (8, 14), (14, 20), (20, 26), (26, 32), (32, PBLK)]
GPS_ZS = 0                     # chunks of zs = zo*sqe computed on GPSIMD
WARMUP_PER_CHUNK = 3           # junk matmuls to warm the PE HAM throttle

# --- host-side packing maps -------------------------------------------------
def _build_maps():
    row_of_block = [(i, 125 - i) for i in range(63)] + [(126,), (127,)]
    slot_row = np.full(PACK, -1, np.int64)
    slot_col = np.full(PACK, -1, np.int64)
    for b, rows in enumerate(row_of_block):
        pos = b * 128
        for r in rows:
            L = r + 1
            slot_row[pos:pos + L] = r
            slot_col[pos:pos + L - 1] = np.arange(r)
            slot_col[pos + L - 1] = r              # diag/sentinel slot
            pos += L + (1 if r % 2 == 0 else 0)    # pad rows to even length
    return slot_row, slot_col

_slot_row, _slot_col = _build_maps()
_valid = _slot_row >= 0
_strict = _valid & (_slot_col < _slot_row)
_diag = _valid & (_slot_col == _slot_row)
_tri_idx = (_slot_row[_strict] * (_slot_row[_strict] - 1) // 2
            + _slot_col[_strict])

def _build_S():
    from ml_dtypes import float8_e4m3
    pr = np.full(PPAD, -1, np.int64)
    pr[:NPAIR] = _slot_row[0::2]
    S = np.zeros((PBLK, 128, 128), np.float32)
    k = np.arange(128)
    for q in range(PBLK):
        s = pr[q * 128:(q + 1) * 128]
        same = (s[:, None] == s[None, :]) & (s[:, None] >= 0)
        S[q] = np.where(same & (k[:, None] < k[None, :]), 1.0, 0.0)
    return np.ascontiguousarray(
        S.transpose(1, 0, 2)).astype(float8_e4m3)   # [k, blk, t]

_S_host = _build_S()

_prog_cache = {}


def _build_program():
    import concourse.bacc as bacc
    import concourse.mybir as mybir
    import bass_rust
    from concourse.tile import TileContext

    f16 = mybir.dt.float16
    f32 = mybir.dt.float32
    f8 = mybir.dt.float8e4
    Act = mybir.ActivationFunctionType
    Alu = mybir.AluOpType

    nc = bacc.Bacc("TRN2", target_bir_lowering=False, debug=False,
                   num_devices=NCORES)

    # The act-table placement pass maps each activation func to the FIRST
    # set containing it (ln->natural_log, exp->exp_and_others), forcing a
    # 1.28us table reload on every ln<->exp transition. Filter the table
    # list it sees so ln/exp resolve only to natural_log_exp_and_others
    # and sqrt to sqrt_and_others; list order (and thus the emitted
    # act_func_set_id indices) is unchanged, so the real tables load.
    import types
    from concourse.hw_specs import get_activation_tables

    def _patched_table_loads(self):
        has_activation = any(
            isinstance(i, mybir.InstActivation)
            for b in self.main_func.blocks
            for i in b.instructions
        )
        if not has_activation:
            return
        keep = {"sqrt_and_others", "natural_log_exp_and_others"}
        drop = {Act.Ln, Act.Exp, Act.Sqrt}
        tables = [
            (name, fns if name in keep else fns - drop)
            for name, fns in get_activation_tables(self.m.arch).items()
        ]
        bass_rust.insert_act_table_loads(self, tables)

    nc.insert_act_table_loads = types.MethodType(_patched_table_loads, nc)
    ze_d = nc.dram_tensor("ze", [128, PBLK, SAMP], f16,
                          kind="ExternalInput").ap()
    zo_d = nc.dram_tensor("zo", [128, PBLK, SAMP], f16,
                          kind="ExternalInput").ap()
    sc_d = nc.dram_tensor("sc", [128, PBLK, 128], f8,
                          kind="ExternalInput").ap()
    lp_d = nc.dram_tensor("lp", [128, PBLK, 2, SAMP], f16,
                          kind="ExternalOutput").ap()

    def load_table(set_id):
        _tl = bass_rust.InstLoadActFuncSet(
            name=nc.get_next_instruction_name(), ins=[], outs=[],
            act_func_set_id=set_id)
        nc.scalar.add_instruction(_tl)

    with TileContext(nc) as tc:
        with (
            tc.tile_pool(name="sb", bufs=1) as sb,
            tc.psum_pool(name="ps", bufs=2) as pp_pool,
            tc.psum_pool(name="junk", bufs=1) as junk_pool,
        ):
            ze = sb.tile([128, PBLK, SAMP], f16)
            zo = sb.tile([128, PBLK, SAMP], f16)
            uez = sb.tile([128, PBLK, SAMP], f16)    # ue, later zs
            uo = sb.tile([128, PBLK, SAMP], f16)
            buw = sb.tile([128, PBLK, SAMP], f16)    # buen, later w
            buon = sb.tile([128, PBLK, SAMP], f16)
            ppt = sb.tile([128, PBLK, SAMP], f16)
            sqe = sb.tile([128, PBLK, SAMP], f16)
            ge = sb.tile([128, PBLK, SAMP], f16)
            lt = sb.tile([128, PBLK, 2, SAMP], f16)
            st = sb.tile([128, PBLK, 128], f8)

            # ---- phase A: ze in, ue, sqe (sqrt table) ----
            for a, b in CHUNKS:
                g = (slice(None), slice(a, b), slice(None))
                nc.sync.dma_start(out=ze[g], in_=ze_d[g])
            jt = junk_pool.tile([128, 2, SAMP], f32, tag="junk")
            for i, (a, b) in enumerate(CHUNKS):
                g = (slice(None), slice(a, b), slice(None))
                nc.vector.tensor_mul(uez[g], ze[g], ze[g])
                nc.scalar.activation(sqe[g], uez[g], Act.Sqrt,
                                     bias=1.0, scale=-1.0)
                # PE warmup: junk matmuls on the freshly-arrived ze chunk
                for k in range(WARMUP_PER_CHUNK):
                    nc.tensor.matmul(jt[:, k % 2, :], ze[:, a, 0:128],
                                     ze[:, a, :])

            # ---- phase B ----
            nc.sync.dma_start(out=st[:], in_=sc_d)
            for a, b in CHUNKS:
                g = (slice(None), slice(a, b), slice(None))
                nc.sync.dma_start(out=zo[g], in_=zo_d[g])

            def emit_batch(a, b):
                nb = b - a
                pt = pp_pool.tile([128, 6, SAMP], f32, tag="ps")
                for j in range(nb):
                    nc.tensor.matmul(pt[:, j, :], st[:, a + j, :],
                                     buw[:, a + j, :])
                gsl = (slice(None), slice(a, b), slice(None))
                nc.scalar.activation(ge[gsl], pt[:, 0:nb, :], Act.Exp,
                                     scale=0.5)
                nc.vector.tensor_mul(lt[:, a:b, 0, :], ze[gsl], ge[gsl])
                nc.vector.tensor_mul(lt[:, a:b, 1, :], uez[gsl], ge[gsl])
                nc.sync.dma_start(out=lp_d[:, a:b, :, :],
                                  in_=lt[:, a:b, :, :])

            prev = None
            for i, (a, b) in enumerate(CHUNKS):
                g = (slice(None), slice(a, b), slice(None))
                nc.vector.tensor_mul(uo[g], zo[g], zo[g])
                nc.vector.tensor_scalar(buw[g], uez[g], 1.0, None,
                                        Alu.subtract)        # ue - 1
                nc.vector.tensor_scalar(buon[g], uo[g], 1.0, None,
                                        Alu.subtract)        # uo - 1
                nc.vector.tensor_mul(ppt[g], buw[g], buon[g])
                # zs = zo*sqe overwrites ue (dead after buen/sqe)
                if i < GPS_ZS:
                    nc.gpsimd.tensor_tensor(uez[g], zo[g], sqe[g], Alu.mult)
                else:
                    nc.vector.tensor_mul(uez[g], zo[g], sqe[g])
                nc.scalar.activation(buw[g], ppt[g], Act.Ln)  # w over buen
                if prev is not None:
                    emit_batch(*prev)
                prev = (a, b)
            emit_batch(*prev)
    nc.compile()
    return nc


def _get_program():
    if "nc" not in _prog_cache:
        _prog_cache["nc"] = _build_program()
    return _prog_cache["nc"]


def _to_core(a):
    # [SAMP, PPAD] -> [128, PBLK, SAMP]
    return np.ascontiguousarray(
        a.T.reshape(PBLK, 128, SAMP).transpose(1, 0, 2))


def kernel(inputs: np.ndarray, _return_raw=False, **run_kw) -> np.ndarray:
    from concourse.bass_utils import run_bass_kernel_spmd

    assert inputs.shape == (B, NZ), inputs.shape
    zvec = inputs.astype(np.float16)

    zpk = np.zeros((B, PACK), np.float16)
    zpk[:, _strict] = zvec[:, _tri_idx]
    zpk[:, _diag] = SENT
    ze_all = np.zeros((B, PPAD), np.float16)
    zo_all = np.zeros((B, PPAD), np.float16)
    ze_all[:, :NPAIR] = zpk[:, 0::2]
    zo_all[:, :NPAIR] = zpk[:, 1::2]

    in_maps = []
    for c in range(NCORES):
        sl = slice(c * SAMP, (c + 1) * SAMP)
        in_maps.append({"ze": _to_core(ze_all[sl]),
                        "zo": _to_core(zo_all[sl]),
                        "sc": _S_host})

    nc = _get_program()
    res = run_bass_kernel_spmd(nc, in_maps, list(range(NCORES)), **run_kw)

    lpk = np.zeros((B, PACK), np.float32)
    for c in range(NCORES):
        lc = res.results[c]["lp"]               # [128, PBLK, 2, SAMP]
        le = lc[:, :, 0, :].transpose(1, 0, 2).reshape(PPAD, SAMP).T
        lo = lc[:, :, 1, :].transpose(1, 0, 2).reshape(PPAD, SAMP).T
        sl = slice(c * SAMP, (c + 1) * SAMP)
        lpk[sl, 0::2] = le[:, :NPAIR]
        lpk[sl, 1::2] = lo[:, :NPAIR]

    out = np.zeros((B, N, N), np.float32)
    out[:, _slot_row[_strict], _slot_col[_strict]] = lpk[:, _strict]
    out[:, _slot_row[_diag], _slot_col[_diag]] = (
        lpk[:, _diag] / np.float32(SENT))
    if _return_raw:
        return out, res
    return out


# revision 16
# speedup vs baseline: 1.0722x; 1.0265x over previous
"""Trainium2 Bass kernel for nn_Cholesky_from_z (pair-compressed, merged-ln).

Closed form: L[i,j] = z[i,j] * sqrt(prod_{k<j}(1-z[i,k]^2)) (j<i),
L[i,i] = sqrt(prod_{k<i}(...)) -- exclusive cumprod of a=(1-z^2) along each
matrix row, with the serial scan replaced by a matmul on the tensor engine
and positions PAIR-COMPRESSED.

v6 structure (one table swap, merged single Ln):
  phase A [sqrt table]:  ue = ze*ze (DVE); sqe = Sqrt(1-ue) (ACT)
  swap to ln/exp table
  phase B: uo = zo*zo; buen = ue-1; buon = uo-1; pp = buen*buon   DVE
           w  = Ln(pp)          ACT   (pp = (1-ze^2)(1-zo^2) >= 0)
           C  = S^T @ w         PE    (S fp8 {0,1}, per pair-block)
           ge = Exp(0.5*C)      ACT
           zs = zo*sqe; le = ze*ge; lo = zs*ge                    DVE
Diagonal sentinel z=0.998 (keeps Ln finite); host divides diag by it.
Batch 2048 sharded 256 samples/core over 8 cores; fp16 I/O.
PE is pre-warmed with junk matmuls during phase A so the HAM throttle is
at full rate when the real matmuls arrive.
"""

import sys

if "/opt/trn_rl_repo" not in sys.path:
    sys.path.insert(0, "/opt/trn_rl_repo")

import numpy as np

B = 2048
N = 128
NZ = N * (N - 1) // 2          # 8128
NBLK = 65
PACK = NBLK * 128              # 8320 (even-aligned rows)
NPAIR = PACK // 2              # 4160
PBLK = 33
PPAD = PBLK * 128              # 4224
NCORES = 8
SAMP = B // NCORES             # 256
SENT = np.float16(0.998)
CHUNKS = [(0, 6), (6, 12), (12, 18), (18, 24), (24, 30), (30, PBLK)]
WARMUP_PER_CHUNK = 3           # junk matmuls to warm the PE HAM throttle


def _register_sqneg_mul():
    """Custom fused DVE op: out = (sq(in0)*s0 + s1) * in1.
    Used as pp = (zo^2 - 1) * (ze^2 - 1), replacing a tensor_tensor
    square, a tensor_scalar subtract, and a tensor_tensor multiply with
    one (1x-rate) instruction. Registered via the documented dve_ops
    extension point."""
    import concourse.dve_ops as dve_ops
    if "SQNEG_MUL" in dve_ops._SUB_OPCODE_FOR_NAME:
        return next(o for o in dve_ops.OPS if o.name == "SQNEG_MUL")
    from concourse.dve_spec import Spec, Src0, Src1, C0, C1, sq, lower
    from concourse.dve_uop import DveOpSpec

    spec = Spec(body=(sq(Src0) * C0 + C1) * Src1)
    row = dve_ops._CUSTOM_DVE_ROW_BASE + len(dve_ops.OPS)
    assert row < 0x20
    shas = {}
    for ver in ("v3", "v4"):
        uops = lower(spec, ver=ver)
        shas[ver] = DveOpSpec(name="SQNEG_MUL", opcode=row, uops=uops,
                              rd1_en=True).sha(ver)
    op = dve_ops.DveOp("SQNEG_MUL", spec, subdim=False, uops_sha=shas)
    dve_ops.OPS.append(op)
    dve_ops._SUB_OPCODE_FOR_NAME["SQNEG_MUL"] = row
    dve_ops.CUSTOM_DVE_SPECS["SQNEG_MUL"] = spec
    return op

# --- host-side packing maps -------------------------------------------------
def _build_maps():
    row_of_block = [(i, 125 - i) for i in range(63)] + [(126,), (127,)]
    slot_row = np.full(PACK, -1, np.int64)
    slot_col = np.full(PACK, -1, np.int64)
    for b, rows in enumerate(row_of_block):
        pos = b * 128
        for r in rows:
            L = r + 1
            slot_row[pos:pos + L] = r
            slot_col[pos:pos + L - 1] = np.arange(r)
            slot_col[pos + L - 1] = r              # diag/sentinel slot
            pos += L + (1 if r % 2 == 0 else 0)    # pad rows to even length
    return slot_row, slot_col

_slot_row, _slot_col = _build_maps()
_valid = _slot_row >= 0
_strict = _valid & (_slot_col < _slot_row)
_diag = _valid & (_slot_col == _slot_row)
_tri_idx = (_slot_row[_strict] * (_slot_row[_strict] - 1) // 2
            + _slot_col[_strict])

def _build_S():
    from ml_dtypes import float8_e4m3
    pr = np.full(PPAD, -1, np.int64)
    pr[:NPAIR] = _slot_row[0::2]
    S = np.zeros((PBLK, 128, 128), np.float32)
    k = np.arange(128)
    for q in range(PBLK):
        s = pr[q * 128:(q + 1) * 128]
        same = (s[:, None] == s[None, :]) & (s[:, None] >= 0)
        S[q] = np.where(same & (k[:, None] < k[None, :]), 1.0, 0.0)
    return np.ascontiguousarray(
        S.transpose(1, 0, 2)).astype(float8_e4m3)   # [k, blk, t]

_S_host = _build_S()

_prog_cache = {}


def _build_program():
    import concourse.bacc as bacc
    import concourse.mybir as mybir
    import bass_rust
    from concourse.tile import TileContext

    sqm = _register_sqneg_mul()

    f16 = mybir.dt.float16
    f32 = mybir.dt.float32
    f8 = mybir.dt.float8e4
    Act = mybir.ActivationFunctionType
    Alu = mybir.AluOpType

    nc = bacc.Bacc("TRN2", target_bir_lowering=False, debug=False,
                   num_devices=NCORES)

    # The act-table placement pass maps each activation func to the FIRST
    # set containing it (ln->natural_log, exp->exp_and_others), forcing a
    # 1.28us table reload on every ln<->exp transition. Filter the table
    # list it sees so ln/exp resolve only to natural_log_exp_and_others
    # and sqrt to sqrt_and_others; list order (and thus the emitted
    # act_func_set_id indices) is unchanged, so the real tables load.
    import types
    from concourse.hw_specs import get_activation_tables

    def _patched_table_loads(self):
        has_activation = any(
            isinstance(i, mybir.InstActivation)
            for b in self.main_func.blocks
            for i in b.instructions
        )
        if not has_activation:
            return
        keep = {"sqrt_and_others", "natural_log_exp_and_others"}
        drop = {Act.Ln, Act.Exp, Act.Sqrt}
        tables = [
            (name, fns if name in keep else fns - drop)
            for name, fns in get_activation_tables(self.m.arch).items()
        ]
        bass_rust.insert_act_table_loads(self, tables)

    nc.insert_act_table_loads = types.MethodType(_patched_table_loads, nc)
    ze_d = nc.dram_tensor("ze", [128, PBLK, SAMP], f16,
                          kind="ExternalInput").ap()
    zo_d = nc.dram_tensor("zo", [128, PBLK, SAMP], f16,
                          kind="ExternalInput").ap()
    sc_d = nc.dram_tensor("sc", [128, PBLK, 128], f8,
                          kind="ExternalInput").ap()
    lp_d = nc.dram_tensor("lp", [128, PBLK, 2, SAMP], f16,
                          kind="ExternalOutput").ap()

    def load_table(set_id):
        _tl = bass_rust.InstLoadActFuncSet(
            name=nc.get_next_instruction_name(), ins=[], outs=[],
            act_func_set_id=set_id)
        nc.scalar.add_instruction(_tl)

    with TileContext(nc) as tc:
        with (
            tc.tile_pool(name="sb", bufs=1) as sb,
            tc.psum_pool(name="ps", bufs=2) as pp_pool,
            tc.psum_pool(name="junk", bufs=1) as junk_pool,
        ):
            ze = sb.tile([128, PBLK, SAMP], f16)
            zo = sb.tile([128, PBLK, SAMP], f16)
            uez = sb.tile([128, PBLK, SAMP], f16)    # ue, later zs
            buw = sb.tile([128, PBLK, SAMP], f16)    # buen, later w
            ppt = sb.tile([128, PBLK, SAMP], f16)
            sqe = sb.tile([128, PBLK, SAMP], f16)
            ge = sb.tile([128, PBLK, SAMP], f16)
            lt = sb.tile([128, PBLK, 2, SAMP], f16)
            st = sb.tile([128, PBLK, 128], f8)

            # ---- phase A: ze in, ue, sqe (sqrt table) ----
            for a, b in CHUNKS:
                g = (slice(None), slice(a, b), slice(None))
                nc.sync.dma_start(out=ze[g], in_=ze_d[g])
            jt = junk_pool.tile([128, 2, SAMP], f32, tag="junk")
            for i, (a, b) in enumerate(CHUNKS):
                g = (slice(None), slice(a, b), slice(None))
                nc.vector.tensor_mul(uez[g], ze[g], ze[g])
                nc.scalar.activation(sqe[g], uez[g], Act.Sqrt,
                                     bias=1.0, scale=-1.0)
                nc.vector.tensor_scalar(buw[g], uez[g], 1.0, None,
                                        Alu.subtract)        # buen = ue - 1
                # PE warmup: junk matmuls on the freshly-arrived ze chunk
                for k in range(WARMUP_PER_CHUNK):
                    nc.tensor.matmul(jt[:, k % 2, :], ze[:, a, 0:128],
                                     ze[:, a, :])

            # ---- phase B ----
            nc.sync.dma_start(out=st[:], in_=sc_d)
            for a, b in CHUNKS:
                g = (slice(None), slice(a, b), slice(None))
                nc.sync.dma_start(out=zo[g], in_=zo_d[g])

            def emit_batch(a, b):
                nb = b - a
                pt = pp_pool.tile([128, 6, SAMP], f32, tag="ps")
                for j in range(nb):
                    nc.tensor.matmul(pt[:, j, :], st[:, a + j, :],
                                     buw[:, a + j, :])
                gsl = (slice(None), slice(a, b), slice(None))
                nc.scalar.activation(ge[gsl], pt[:, 0:nb, :], Act.Exp,
                                     scale=0.5)
                nc.vector.tensor_mul(lt[:, a:b, 0, :], ze[gsl], ge[gsl])
                nc.vector.tensor_mul(lt[:, a:b, 1, :], uez[gsl], ge[gsl])
                nc.sync.dma_start(out=lp_d[:, a:b, :, :],
                                  in_=lt[:, a:b, :, :])

            prev = None
            for i, (a, b) in enumerate(CHUNKS):
                g = (slice(None), slice(a, b), slice(None))
                # pp = (zo^2 - 1) * buen, fused in one DVE op
                nc.vector._custom_dve(sqm, out=ppt[g], in0=zo[g],
                                      in1=buw[g], s0=1.0, s1=-1.0)
                # zs = zo*sqe overwrites ue (dead after buen/sqe)
                nc.vector.tensor_mul(uez[g], zo[g], sqe[g])
                nc.scalar.activation(buw[g], ppt[g], Act.Ln)  # w over buen
                if prev is not None:
                    emit_batch(*prev)
                prev = (a, b)
            emit_batch(*prev)
    nc.compile()
    return nc


def _get_program():
    if "nc" not in _prog_cache:
        _prog_cache["nc"] = _build_program()
    return _prog_cache["nc"]


def _to_core(a):
    # [SAMP, PPAD] -> [128, PBLK, SAMP]
    return np.ascontiguousarray(
        a.T.reshape(PBLK, 128, SAMP).transpose(1, 0, 2))


def kernel(inputs: np.ndarray, _return_raw=False, **run_kw) -> np.ndarray:
    from concourse.bass_utils import run_bass_kernel_spmd

    assert inputs.shape == (B, NZ), inputs.shape
    zvec = inputs.astype(np.float16)

    zpk = np.zeros((B, PACK), np.float16)
    zpk[:, _strict] = zvec[:, _tri_idx]
    zpk[:, _diag] = SENT
    ze_all = np.zeros((B, PPAD), np.float16)
    zo_all = np.zeros((B, PPAD), np.float16)
    ze_all[:, :NPAIR] = zpk[:, 0::2]
    zo_all[:, :NPAIR] = zpk[:, 1::2]

    in_maps = []
    for c in range(NCORES):
        sl = slice(c * SAMP, (c + 1) * SAMP)
        in_maps.append({"ze": _to_core(ze_all[sl]),
                        "zo": _to_core(zo_all[sl]),
                        "sc": _S_host})

    nc = _get_program()
    res = run_bass_kernel_spmd(nc, in_maps, list(range(NCORES)), **run_kw)

    lpk = np.zeros((B, PACK), np.float32)
    for c in range(NCORES):
        lc = res.results[c]["lp"]               # [128, PBLK, 2, SAMP]
        le = lc[:, :, 0, :].transpose(1, 0, 2).reshape(PPAD, SAMP).T
        lo = lc[:, :, 1, :].transpose(1, 0, 2).reshape(PPAD, SAMP).T
        sl = slice(c * SAMP, (c + 1) * SAMP)
        lpk[sl, 0::2] = le[:, :NPAIR]
        lpk[sl, 1::2] = lo[:, :NPAIR]

    out = np.zeros((B, N, N), np.float32)
    out[:, _slot_row[_strict], _slot_col[_strict]] = lpk[:, _strict]
    out[:, _slot_row[_diag], _slot_col[_diag]] = (
        lpk[:, _diag] / np.float32(SENT))
    if _return_raw:
        return out, res
    return out
